# revision 5
# baseline (speedup 1.0000x reference)
"""Trainium2 Bass kernel: coverage-attention LSTM decoder scan.

Reference math (per step, torch gate order i,f,g,o):
    gates = s @ W_ih.T + b_ih + h @ W_hh.T + b_hh
    c' = sig(f)*c + sig(i)*tanh(g);  h' = sig(o)*tanh(c')
    dec = [h';c'] @ W_x
    feat = tanh(z_fea + dec[:,None,:] + coverage[...,None]*w_c + b_attn)
    e = feat @ v ; attn = softmax(e); c_t = attn @ z
    closs += mean_b sum_n min(attn, coverage); coverage += attn

Sharding: data-parallel over batch (32 -> 4 per core x 8 cores), no
collectives.  Per core everything stays resident in SBUF across the
64-step scan.

Layouts per core (b = local batch 0..3, n = c*128+p):
    zf      [128 a, b*4096+n]   f32   z @ W_z, A on partitions
    znb     [128 p, (b*32+c)*128+d] bf16  z, N on partitions (for c_t)
    cov_col [128 p, b*32+c]     f32   coverage, N on partitions
    cov_row [1, b*4096+n]       bf16  coverage row mirror (gpsimd bcast src)
    e/exp/attn_col same col layout as cov_col
"""

import sys
import functools

sys.path.insert(0, "/opt/trn_rl_repo")
sys.path.insert(0, "/opt/trn_rl_repo/concourse")

import numpy as np
import ml_dtypes

import concourse.bass as bass
import concourse.tile as tile
from concourse import bacc, mybir
from concourse import library_config
from concourse.bass_utils import run_bass_kernel_spmd

B, N, DK, H, A = 32, 4096, 128, 128, 128
NCORES = 8
BL = B // NCORES            # 4 batch items per core
NCH = N // 128              # 32 n-chunks
GRAN = 1024                 # free-dim granule for the big feat pipeline
NG = (BL * N) // GRAN       # 16 granules per step

F32 = mybir.dt.float32
F32R = mybir.dt.float32r
BF16 = mybir.dt.bfloat16
FP16 = mybir.dt.float16
AF = mybir.ActivationFunctionType
OP = mybir.AluOpType
AX = mybir.AxisListType


def _r(ap):
    return ap.bitcast(F32R)


@functools.lru_cache(maxsize=2)
def _build(n_steps: int):
    nc = bacc.Bacc("TRN2", target_bir_lowering=False, debug=False,
                   num_devices=NCORES)

    # ---- I/O ----
    d_zt = nc.declare_dram_parameter("zt", [128, BL * N], F32, isOutput=False)
    d_znb = nc.declare_dram_parameter("znb", [128, BL * N], FP16, isOutput=False)
    d_h0 = nc.declare_dram_parameter("h0t", [H, BL], F32, isOutput=False)
    d_c0 = nc.declare_dram_parameter("c0t", [H, BL], F32, isOutput=False)
    d_wih = nc.declare_dram_parameter("wihT", [DK, 4 * H], F32, isOutput=False)
    d_whh = nc.declare_dram_parameter("whhT", [H, 4 * H], F32, isOutput=False)
    d_wxh = nc.declare_dram_parameter("wxh", [H, A], F32, isOutput=False)
    d_wxc = nc.declare_dram_parameter("wxc", [H, A], F32, isOutput=False)
    d_wz = nc.declare_dram_parameter("wz", [DK, A], F32, isOutput=False)
    d_wc = nc.declare_dram_parameter("wc", [A, 1], F32, isOutput=False)
    d_v = nc.declare_dram_parameter("v", [A, 1], F32, isOutput=False)
    d_v16 = nc.declare_dram_parameter("v16", [A, 1], FP16, isOutput=False)
    d_batt = nc.declare_dram_parameter("batt", [A, 1], F32, isOutput=False)
    d_bg = nc.declare_dram_parameter("bg", [H, 4], F32, isOutput=False)

    d_attn = nc.dram_tensor("attn_out", [n_steps, 128, BL * NCH], F32,
                            kind="ExternalOutput")
    d_text = nc.dram_tensor("text_out", [n_steps, 128, BL], F32,
                            kind="ExternalOutput")
    d_acc = nc.dram_tensor("acc_out", [128, BL * NCH], F32,
                           kind="ExternalOutput")
    d_bcov = nc.dram_tensor("bounce_cov", [BL, 128 * NCH], FP16)
    d_bct = nc.dram_tensor("bounce_ct", [BL, DK], F32)

    with tile.TileContext(nc) as tc:
        with tc.tile_pool(name="pers", bufs=1) as pers:
            zf = pers.tile([128, BL * N], F32, tag="zf")
            znb = pers.tile([128, BL * N], FP16, tag="znb")
            cov_col = pers.tile([128, BL * NCH], F32, tag="cov_col")
            covb16 = pers.tile([128, BL * NCH], FP16, tag="covb16")
            cov_row = pers.tile([1, BL * N], FP16, tag="cov_row")
            acc = pers.tile([128, BL * NCH], F32, tag="acc")
            e_col = pers.tile([128, BL * NCH], F32, tag="e_col")
            exp_col = pers.tile([128, BL * NCH], F32, tag="exp_col")
            expb16 = pers.tile([128, BL * NCH], FP16, tag="expb16")
            attn_col = pers.tile([128, BL * NCH], F32, tag="attn_col")
            hT = pers.tile([H, BL], F32, tag="hT")
            cT = pers.tile([H, BL], F32, tag="cT")
            sT = pers.tile([DK, BL], F32, tag="sT")
            sT_raw = pers.tile([DK, BL], F32, tag="sT_raw")
            wih = pers.tile([DK, 4 * H], F32, tag="wih")
            whh = pers.tile([H, 4 * H], F32, tag="whh")
            wxh = pers.tile([H, A], F32, tag="wxh")
            wxc = pers.tile([H, A], F32, tag="wxc")
            wc = pers.tile([A, 1], F32, tag="wc")
            vv = pers.tile([A, 1], F32, tag="vv")
            v16 = pers.tile([A, 1], FP16, tag="v16")
            batt = pers.tile([A, 1], F32, tag="batt")
            bg = pers.tile([H, 4], F32, tag="bg")
            ones = pers.tile([128, 1], F32, tag="ones")
            s_sb = pers.tile([1, BL], F32, tag="s_sb")
            recipS = pers.tile([1, BL], F32, tag="recipS")
            recipS_bc = pers.tile([128, BL], F32, tag="recipS_bc")
            dec_sb = pers.tile([A, BL], F32, tag="dec_sb")

            # ---- setup: load weights/state, precompute zf ----
            nc.gpsimd.load_library(library_config.attn)
            for dst, src in [(znb, d_znb), (hT, d_h0), (cT, d_c0),
                             (wih, d_wih), (whh, d_whh), (wxh, d_wxh),
                             (wxc, d_wxc), (wc, d_wc), (vv, d_v), (v16, d_v16),
                             (batt, d_batt), (bg, d_bg)]:
                nc.sync.dma_start(out=dst[:], in_=src[:])
            nc.vector.memset(cov_col[:], 0.0)
            nc.vector.memset(cov_row[:], 0.0)
            nc.vector.memset(acc[:], 0.0)
            nc.vector.memset(sT[:], 0.0)
            nc.vector.memset(ones[:], 1.0)

            with tc.tile_pool(name="zstream", bufs=3) as zpool, \
                 tc.tile_pool(name="ps_z", bufs=2, space="PSUM") as ps_z:
                wz_sb = zpool.tile([DK, A], F32, tag="wz")
                nc.sync.dma_start(out=wz_sb[:], in_=d_wz[:])
                for s in range((BL * N) // 512):
                    sl = slice(s * 512, (s + 1) * 512)
                    ztt = zpool.tile([128, 512], F32, tag="ztt")
                    nc.sync.dma_start(out=ztt[:], in_=d_zt[:, sl])
                    pz = ps_z.tile([128, 512], F32, tag="pz")
                    nc.tensor.matmul(pz[:], wz_sb[:], ztt[:],
                                     start=True, stop=True)
                    nc.scalar.copy(out=zf[:, sl], in_=pz[:])

            # ---- the scan ----
            with tc.tile_pool(name="gp", bufs=2) as gpool, \
                 tc.tile_pool(name="tp", bufs=2) as tpool, \
                 tc.tile_pool(name="fp", bufs=3) as fpool, \
                 tc.tile_pool(name="sm", bufs=4) as small, \
                 tc.tile_pool(name="ps_sm", bufs=2, space="PSUM") as ps_sm, \
                 tc.tile_pool(name="ps_e", bufs=3, space="PSUM") as ps_e, \
                 tc.tile_pool(name="ps_ct", bufs=2, space="PSUM") as ps_ct:

                for t in range(n_steps):
                    # -- LSTM cell --
                    gact = []
                    for g in range(4):
                        pg = ps_sm.tile([H, BL], F32, tag="ps")
                        gs = slice(g * H, (g + 1) * H)
                        nc.tensor.matmul(pg[:], wih[:, gs], sT[:],
                                         start=True, stop=False)
                        nc.tensor.matmul(pg[:], whh[:, gs], hT[:],
                                         start=False, stop=True)
                        ga = small.tile([H, BL], F32, tag=f"g{g}")
                        # i,f,o: sigmoid(x)=0.5*(1+tanh(x/2)); g: tanh(x)
                        sc = 1.0 if g == 2 else 0.5
                        nc.scalar.activation(ga[:], pg[:], AF.Tanh,
                                             bias=bg[:, g:g + 1], scale=sc)
                        gact.append(ga)
                    ti, tf, tg, to = gact
                    u1 = small.tile([H, BL], F32, tag="u1")
                    nc.vector.scalar_tensor_tensor(u1[:], tf[:], 1.0, cT[:],
                                                   OP.add, OP.mult)
                    u2 = small.tile([H, BL], F32, tag="u2")
                    nc.vector.scalar_tensor_tensor(u2[:], ti[:], 1.0, tg[:],
                                                   OP.add, OP.mult)
                    s2 = small.tile([H, BL], F32, tag="s2")  # = 2*c_new
                    nc.vector.tensor_tensor(s2[:], u1[:], u2[:], OP.add)
                    nc.vector.tensor_scalar(cT[:], s2[:], 0.5, None, OP.mult)
                    tc_ = small.tile([H, BL], F32, tag="tc_")  # tanh(c_new)
                    nc.scalar.activation(tc_[:], s2[:], AF.Tanh, scale=0.5)
                    u3 = small.tile([H, BL], F32, tag="u3")  # = 2*h_new
                    nc.vector.scalar_tensor_tensor(u3[:], to[:], 1.0, tc_[:],
                                                   OP.add, OP.mult)
                    nc.vector.tensor_scalar(hT[:], u3[:], 0.5, None, OP.mult)
                    pdec = ps_sm.tile([A, BL], F32, tag="ps")
                    nc.tensor.matmul(pdec[:], wxh[:], hT[:],
                                     start=True, stop=False)
                    nc.tensor.matmul(pdec[:], wxc[:], cT[:],
                                     start=False, stop=True)
                    nc.vector.tensor_scalar(dec_sb[:], pdec[:],
                                            batt[:, 0:1], None, OP.add)

                    # -- feat pipeline + e + softmax + c_t, per batch item --
                    for b in range(4):
                        for g in range(NG // 4):
                            n0 = b * N + g * GRAN
                            cb = gpool.tile([128, GRAN], FP16, tag="cb")
                            nc.gpsimd.partition_broadcast(
                                cb[:], cov_row[0:1, n0:n0 + GRAN])
                            tin = tpool.tile([128, GRAN], F32, tag="tin")
                            nc.vector.scalar_tensor_tensor(
                                tin[:], cb[:], wc[:, 0:1], zf[:, n0:n0 + GRAN],
                                OP.mult, OP.add)
                            feat = fpool.tile([128, GRAN], FP16, tag="feat")
                            nc.scalar.activation(feat[:], tin[:], AF.Tanh,
                                                 bias=dec_sb[:, b:b + 1])
                            nch_g = GRAN // 128
                            pe = ps_e.tile([128, nch_g], F32, tag="pe")
                            for k in range(nch_g):
                                nc.tensor.matmul(
                                    pe[:, k:k + 1],
                                    feat[:, k * 128:(k + 1) * 128], v16[:],
                                    start=True, stop=True)
                            col0 = b * NCH + g * nch_g
                            nc.vector.tensor_copy(
                                out=e_col[:, col0:col0 + nch_g], in_=pe[:])
                        bs = slice(b * NCH, (b + 1) * NCH)
                        nc.scalar.activation(exp_col[:, bs], e_col[:, bs],
                                             AF.Exp)
                        pS = ps_e.tile([128, NCH], F32, tag="pe")
                        nc.tensor.matmul(pS[0:1, :], ones[:],
                                         exp_col[:, bs], start=True, stop=True)
                        nc.vector.tensor_reduce(s_sb[0:1, b:b + 1],
                                                pS[0:1, :], AX.X, OP.add)
                        # c_t accumulation (unnormalized, bf16 weights)
                        nc.vector.tensor_copy(out=expb16[:, bs],
                                              in_=exp_col[:, bs])
                        pct = ps_ct.tile([1, DK], F32, tag="pct")
                        for c in range(NCH):
                            nc.tensor.matmul(
                                pct[:], expb16[:, b * NCH + c:b * NCH + c + 1],
                                znb[:, (b * NCH + c) * 128:
                                        (b * NCH + c + 1) * 128],
                                start=(c == 0), stop=(c == NCH - 1))
                        ct_row = small.tile([1, DK], F32, tag="ct_row")
                        nc.vector.tensor_copy(out=ct_row[:], in_=pct[:])
                        nc.sync.dma_start(out=d_bct[b], in_=ct_row[:])
                        nc.sync.dma_start(out=sT_raw[:, b:b + 1],
                                          in_=d_bct[b])

                    nc.vector.reciprocal(recipS[:], s_sb[:])
                    nc.gpsimd.partition_broadcast(recipS_bc[:], recipS[:])
                    for b in range(4):
                        bs = slice(b * NCH, (b + 1) * NCH)
                        nc.vector.tensor_scalar(attn_col[:, bs],
                                                exp_col[:, bs],
                                                recipS_bc[:, b:b + 1], None,
                                                OP.mult)
                    # closs accumulation + coverage update
                    mt = small.tile([128, BL * NCH], F32, tag="mt")
                    nc.vector.tensor_tensor(mt[:], attn_col[:], cov_col[:],
                                            OP.min)
                    nc.vector.tensor_tensor(acc[:], acc[:], mt[:], OP.add)
                    nc.vector.tensor_tensor(cov_col[:], cov_col[:],
                                            attn_col[:], OP.add)
                    nc.vector.tensor_copy(out=covb16[:], in_=cov_col[:])
                    for b in range(4):
                        nc.sync.dma_start(
                            out=d_bcov[b],
                            in_=covb16[:, b * NCH:(b + 1) * NCH])
                        nc.sync.dma_start(
                            out=cov_row[0:1, b * N:(b + 1) * N],
                            in_=d_bcov[b].rearrange("(p c) -> c p", c=NCH))
                    # context vector -> s for next step; outputs
                    nc.vector.tensor_tensor(sT[:], sT_raw[:], recipS_bc[:],
                                            OP.mult)
                    nc.sync.dma_start(out=d_attn[t], in_=attn_col[:])
                    nc.sync.dma_start(out=d_text[t], in_=sT[:])

            nc.sync.dma_start(out=d_acc[:], in_=acc[:])

    nc.compile()
    return nc


def _prep_inputs(z, h0, c0, W_ih, W_hh, b_ih, b_hh, W_x, W_z, w_c, b_attn, v):
    shared = {
        "wihT": np.ascontiguousarray(np.asarray(W_ih, np.float32).T),
        "whhT": np.ascontiguousarray(np.asarray(W_hh, np.float32).T),
        "wxh": np.ascontiguousarray(np.asarray(W_x, np.float32)[:H]),
        "wxc": np.ascontiguousarray(np.asarray(W_x, np.float32)[H:]),
        "wz": np.ascontiguousarray(np.asarray(W_z, np.float32)),
        "wc": np.asarray(w_c, np.float32).reshape(A, 1),
        "v": np.asarray(v, np.float32).reshape(A, 1),
        "v16": np.asarray(v, np.float16).reshape(A, 1),
        "batt": np.asarray(b_attn, np.float32).reshape(A, 1),
    }
    bgf = (np.asarray(b_ih, np.float32) + np.asarray(b_hh, np.float32))
    bg = np.ascontiguousarray(bgf.reshape(4, H).T)  # [H, 4] cols i,f,g,o
    bg = bg * np.array([0.5, 0.5, 1.0, 0.5], np.float32)[None, :]
    shared["bg"] = np.ascontiguousarray(bg)

    z = np.asarray(z, np.float32)
    h0 = np.asarray(h0, np.float32)
    c0 = np.asarray(c0, np.float32)
    in_maps = []
    for ci in range(NCORES):
        bsl = slice(ci * BL, (ci + 1) * BL)
        zc = z[bsl]                                    # [BL, N, DK]
        zt = np.ascontiguousarray(
            zc.transpose(2, 0, 1).reshape(128, BL * N))
        znb = np.ascontiguousarray(
            zc.reshape(BL, NCH, 128, DK).transpose(2, 0, 1, 3)
              .reshape(128, BL * N)).astype(np.float16)
        m = dict(shared)
        m["zt"] = zt
        m["znb"] = znb
        m["h0t"] = np.ascontiguousarray(h0[bsl].T)
        m["c0t"] = np.ascontiguousarray(c0[bsl].T)
        in_maps.append(m)
    return in_maps


def kernel(z, mask, h0, c0, W_ih, W_hh, b_ih, b_hh, W_x, W_z, w_c, b_attn, v,
           n_node, _trace=False):
    n_steps = int(n_node)
    nc = _build(n_steps)
    in_maps = _prep_inputs(z, h0, c0, W_ih, W_hh, b_ih, b_hh, W_x, W_z,
                           w_c, b_attn, v)
    res = run_bass_kernel_spmd(nc, in_maps, list(range(NCORES)),
                               trace=_trace)
    text = np.empty((B, n_steps, DK), np.float32)
    attns = np.empty((B, n_steps, N), np.float32)
    closs = 0.0
    for ci, r in enumerate(res.results):
        bsl = slice(ci * BL, (ci + 1) * BL)
        text[bsl] = r["text_out"].transpose(2, 0, 1)       # [BL, T, DK]
        a4 = r["attn_out"].reshape(n_steps, 128, BL, NCH)  # [t, p, b, c]
        attns[bsl] = a4.transpose(2, 0, 3, 1).reshape(BL, n_steps, N)
        closs += r["acc_out"].astype(np.float64).sum()
    closs = np.float32(closs / B)
    kernel._last_result = res
    return text, attns, closs


# revision 7
# speedup vs baseline: 4.7057x; 4.7057x over previous
"""Trainium2 Bass kernel: coverage-attention LSTM decoder scan.

Reference math (per step, torch gate order i,f,g,o):
    gates = s @ W_ih.T + b_ih + h @ W_hh.T + b_hh
    c' = sig(f)*c + sig(i)*tanh(g);  h' = sig(o)*tanh(c')
    dec = [h';c'] @ W_x
    feat = tanh(z_fea + dec[:,None,:] + coverage[...,None]*w_c + b_attn)
    e = feat @ v ; attn = softmax(e); c_t = attn @ z
    closs += mean_b sum_n min(attn, coverage); coverage += attn

Sharding: data-parallel over batch (32 -> 4 per core x 8 cores), no
collectives.  Per core everything stays resident in SBUF across the
64-step scan.

Layouts per core (b = local batch 0..3, n = c*128+p):
    zf      [128 a, b*4096+n]   f32   z @ W_z, A on partitions
    znb     [128 p, (b*32+c)*128+d] bf16  z, N on partitions (for c_t)
    cov_col [128 p, b*32+c]     f32   coverage, N on partitions
    cov_row [1, b*4096+n]       bf16  coverage row mirror (gpsimd bcast src)
    e/exp/attn_col same col layout as cov_col
"""

import sys
import functools

sys.path.insert(0, "/opt/trn_rl_repo")
sys.path.insert(0, "/opt/trn_rl_repo/concourse")

import numpy as np
import ml_dtypes

import concourse.bass as bass
import concourse.tile as tile
from concourse import bacc, mybir
from concourse import library_config
from concourse.bass_utils import run_bass_kernel_spmd

B, N, DK, H, A = 32, 4096, 128, 128, 128
NCORES = 8
BL = B // NCORES            # 4 batch items per core
NCH = N // 128              # 32 n-chunks
GRAN = 2048                 # free-dim granule for the big feat pipeline
NG = (BL * N) // GRAN       # 16 granules per step

F32 = mybir.dt.float32
F32R = mybir.dt.float32r
BF16 = mybir.dt.bfloat16
FP16 = mybir.dt.float16
AF = mybir.ActivationFunctionType
OP = mybir.AluOpType
AX = mybir.AxisListType


def _r(ap):
    return ap.bitcast(F32R)


@functools.lru_cache(maxsize=2)
def _build(n_steps: int):
    nc = bacc.Bacc("TRN2", target_bir_lowering=False, debug=False,
                   num_devices=NCORES)

    # ---- I/O ----
    d_zt = nc.declare_dram_parameter("zt", [128, BL * N], F32, isOutput=False)
    d_znb = nc.declare_dram_parameter("znb", [128, BL * N], FP16, isOutput=False)
    d_h0 = nc.declare_dram_parameter("h0t", [H, BL], F32, isOutput=False)
    d_c0 = nc.declare_dram_parameter("c0t", [H, BL], F32, isOutput=False)
    d_wih = nc.declare_dram_parameter("wihT", [DK, 4 * H], F32, isOutput=False)
    d_whh = nc.declare_dram_parameter("whhT", [H, 4 * H], F32, isOutput=False)
    d_wxh = nc.declare_dram_parameter("wxh", [H, A], F32, isOutput=False)
    d_wxc = nc.declare_dram_parameter("wxc", [H, A], F32, isOutput=False)
    d_wz = nc.declare_dram_parameter("wz", [DK, A], F32, isOutput=False)
    d_wc = nc.declare_dram_parameter("wc", [A, 1], F32, isOutput=False)
    d_v = nc.declare_dram_parameter("v", [A, 1], F32, isOutput=False)
    d_v16 = nc.declare_dram_parameter("v16", [A, 1], FP16, isOutput=False)
    d_batt = nc.declare_dram_parameter("batt", [A, 1], F32, isOutput=False)
    d_bg = nc.declare_dram_parameter("bg", [H, 4], F32, isOutput=False)

    d_attn = nc.dram_tensor("attn_out", [n_steps, 128, BL * NCH], F32,
                            kind="ExternalOutput")
    d_text = nc.dram_tensor("text_out", [128, n_steps * BL], F32,
                            kind="ExternalOutput")
    d_acc = nc.dram_tensor("acc_out", [128, BL * NCH], F32,
                           kind="ExternalOutput")
    d_covrow = nc.dram_tensor("covrow", [BL * N], FP16)

    with tile.TileContext(nc) as tc:
        with tc.tile_pool(name="pers", bufs=1) as pers:
            zf = pers.tile([128, BL * N], FP16, tag="zf")
            znb = pers.tile([128, BL * N], FP16, tag="znb")
            cov_col = pers.tile([128, BL * NCH], F32, tag="cov_col")
            covb16 = pers.tile([128, BL * NCH], FP16, tag="covb16")
            covT = pers.tile([128, BL * NCH], FP16, tag="covT")
            acc = pers.tile([128, BL * NCH], F32, tag="acc")
            e_col = pers.tile([128, BL * NCH], F32, tag="e_col")
            exp_col = pers.tile([128, BL * NCH], F32, tag="exp_col")
            expb16 = pers.tile([128, BL * NCH], FP16, tag="expb16")
            attn_col = pers.tile([128, BL * NCH], F32, tag="attn_col")
            hT = pers.tile([H, BL], F32, tag="hT")
            cT = pers.tile([H, BL], F32, tag="cT")
            sT = pers.tile([DK, BL], F32, tag="sT")
            sT_raw = pers.tile([DK, BL], F32, tag="sT_raw")
            text_acc = pers.tile([128, n_steps * BL], F32, tag="text_acc")
            wih = pers.tile([DK, 4 * H], F32, tag="wih")
            whh = pers.tile([H, 4 * H], F32, tag="whh")
            wxh = pers.tile([H, A], F32, tag="wxh")
            wxc = pers.tile([H, A], F32, tag="wxc")
            wc = pers.tile([A, 1], F32, tag="wc")
            vv = pers.tile([A, 1], F32, tag="vv")
            v16 = pers.tile([A, 1], FP16, tag="v16")
            batt = pers.tile([A, 1], F32, tag="batt")
            bg = pers.tile([H, 4], F32, tag="bg")
            ones = pers.tile([128, 1], F32, tag="ones")
            s_sb = pers.tile([1, BL], F32, tag="s_sb")
            recipS = pers.tile([1, BL], F32, tag="recipS")
            recipS_bc = pers.tile([128, BL], F32, tag="recipS_bc")
            dec_sb = pers.tile([A, BL], F32, tag="dec_sb")

            # ---- setup: load weights/state, precompute zf ----
            nc.gpsimd.load_library(library_config.attn)
            for dst, src in [(znb, d_znb), (hT, d_h0), (cT, d_c0),
                             (wih, d_wih), (whh, d_whh), (wxh, d_wxh),
                             (wxc, d_wxc), (wc, d_wc), (vv, d_v), (v16, d_v16),
                             (batt, d_batt), (bg, d_bg)]:
                nc.sync.dma_start(out=dst[:], in_=src[:])
            nc.vector.memset(cov_col[:], 0.0)
            nc.vector.memset(covb16[:], 0.0)
            nc.vector.memset(covT[:], 0.0)
            nc.sync.dma_start(out=d_covrow[:], in_=covT[:])
            nc.vector.memset(acc[:], 0.0)
            nc.vector.memset(sT[:], 0.0)
            nc.vector.memset(ones[:], 1.0)

            with tc.tile_pool(name="zstream", bufs=3) as zpool, \
                 tc.tile_pool(name="ps_z", bufs=2, space="PSUM") as ps_z:
                wz_sb = zpool.tile([DK, A], F32, tag="wz")
                nc.sync.dma_start(out=wz_sb[:], in_=d_wz[:])
                for s in range((BL * N) // 512):
                    sl = slice(s * 512, (s + 1) * 512)
                    ztt = zpool.tile([128, 512], F32, tag="ztt")
                    nc.sync.dma_start(out=ztt[:], in_=d_zt[:, sl])
                    pz = ps_z.tile([128, 512], F32, tag="pz")
                    nc.tensor.matmul(pz[:], wz_sb[:], ztt[:],
                                     start=True, stop=True)
                    nc.scalar.copy(out=zf[:, sl], in_=pz[:])

            # ---- the scan ----
            with tc.tile_pool(name="gp", bufs=2) as gpool, \
                 tc.tile_pool(name="tp", bufs=2) as tpool, \
                 tc.tile_pool(name="fp", bufs=3) as fpool, \
                 tc.tile_pool(name="sm", bufs=4) as small, \
                 tc.tile_pool(name="ps_sm", bufs=2, space="PSUM") as ps_sm, \
                 tc.tile_pool(name="ps_e", bufs=3, space="PSUM") as ps_e, \
                 tc.tile_pool(name="ps_ct", bufs=2, space="PSUM") as ps_ct:

                for t in range(n_steps):
                    # -- LSTM cell --
                    gact = []
                    for g in range(4):
                        pg = ps_sm.tile([H, BL], F32, tag="ps")
                        gs = slice(g * H, (g + 1) * H)
                        nc.tensor.matmul(pg[:], wih[:, gs], sT[:],
                                         start=True, stop=False)
                        nc.tensor.matmul(pg[:], whh[:, gs], hT[:],
                                         start=False, stop=True)
                        ga = small.tile([H, BL], F32, tag=f"g{g}")
                        # i,f,o: sigmoid(x)=0.5*(1+tanh(x/2)); g: tanh(x)
                        sc = 1.0 if g == 2 else 0.5
                        nc.scalar.activation(ga[:], pg[:], AF.Tanh,
                                             bias=bg[:, g:g + 1], scale=sc)
                        gact.append(ga)
                    ti, tf, tg, to = gact
                    u1 = small.tile([H, BL], F32, tag="u1")
                    nc.vector.scalar_tensor_tensor(u1[:], tf[:], 1.0, cT[:],
                                                   OP.add, OP.mult)
                    u2 = small.tile([H, BL], F32, tag="u2")
                    nc.vector.scalar_tensor_tensor(u2[:], ti[:], 1.0, tg[:],
                                                   OP.add, OP.mult)
                    s2 = small.tile([H, BL], F32, tag="s2")  # = 2*c_new
                    nc.vector.tensor_tensor(s2[:], u1[:], u2[:], OP.add)
                    nc.vector.tensor_scalar(cT[:], s2[:], 0.5, None, OP.mult)
                    tc_ = small.tile([H, BL], F32, tag="tc_")  # tanh(c_new)
                    nc.scalar.activation(tc_[:], s2[:], AF.Tanh, scale=0.5)
                    u3 = small.tile([H, BL], F32, tag="u3")  # = 2*h_new
                    nc.vector.scalar_tensor_tensor(u3[:], to[:], 1.0, tc_[:],
                                                   OP.add, OP.mult)
                    nc.vector.tensor_scalar(hT[:], u3[:], 0.5, None, OP.mult)
                    pdec = ps_sm.tile([A, BL], F32, tag="ps")
                    nc.tensor.matmul(pdec[:], wxh[:], hT[:],
                                     start=True, stop=False)
                    nc.tensor.matmul(pdec[:], wxc[:], cT[:],
                                     start=False, stop=True)
                    nc.vector.tensor_scalar(dec_sb[:], pdec[:],
                                            batt[:, 0:1], None, OP.add)

                    # -- feat pipeline + e + softmax + c_t, per batch item --
                    for b in range(4):
                        for g in range(NG // 4):
                            n0 = b * N + g * GRAN
                            cb = gpool.tile([128, GRAN], FP16, tag="cb")
                            csl = d_covrow[n0:n0 + GRAN]
                            csrc = bass.AP(tensor=csl.tensor, offset=csl.offset,
                                           ap=[[0, 128]] + list(csl.ap))
                            eng = nc.sync if g % 2 == 0 else nc.scalar
                            eng.dma_start(out=cb[:], in_=csrc)
                            tin = tpool.tile([128, GRAN], FP16, tag="tin")
                            nc.vector.scalar_tensor_tensor(
                                tin[:], cb[:], wc[:, 0:1], zf[:, n0:n0 + GRAN],
                                OP.mult, OP.add)
                            feat = fpool.tile([128, GRAN], FP16, tag="feat")
                            nc.scalar.activation(feat[:], tin[:], AF.Tanh,
                                                 bias=dec_sb[:, b:b + 1])
                            nch_g = GRAN // 128
                            pe = ps_e.tile([128, nch_g], F32, tag="pe")
                            for k in range(nch_g):
                                nc.tensor.matmul(
                                    pe[:, k:k + 1],
                                    feat[:, k * 128:(k + 1) * 128], v16[:],
                                    start=True, stop=True)
                            col0 = b * NCH + g * nch_g
                            nc.vector.tensor_copy(
                                out=e_col[:, col0:col0 + nch_g], in_=pe[:])
                        bs = slice(b * NCH, (b + 1) * NCH)
                        nc.scalar.activation(exp_col[:, bs], e_col[:, bs],
                                             AF.Exp)
                        pS = ps_e.tile([128, NCH], F32, tag="pe")
                        nc.tensor.matmul(pS[0:1, :], ones[:],
                                         exp_col[:, bs], start=True, stop=True)
                        nc.vector.tensor_reduce(s_sb[0:1, b:b + 1],
                                                pS[0:1, :], AX.X, OP.add)
                        # c_t accumulation (unnormalized, bf16 weights)
                        nc.vector.tensor_copy(out=expb16[:, bs],
                                              in_=exp_col[:, bs])
                        pct = ps_ct.tile([DK, 1], F32, tag="pct")
                        for c in range(NCH):
                            nc.tensor.matmul(
                                pct[:], znb[:, (b * NCH + c) * 128:
                                               (b * NCH + c + 1) * 128],
                                expb16[:, b * NCH + c:b * NCH + c + 1],
                                start=(c == 0), stop=(c == NCH - 1))
                        nc.vector.tensor_copy(out=sT_raw[:, b:b + 1],
                                              in_=pct[:])

                    nc.vector.reciprocal(recipS[:], s_sb[:])
                    nc.gpsimd.partition_broadcast(recipS_bc[:], recipS[:])
                    for b in range(4):
                        bs = slice(b * NCH, (b + 1) * NCH)
                        nc.vector.tensor_scalar(attn_col[:, bs],
                                                exp_col[:, bs],
                                                recipS_bc[:, b:b + 1], None,
                                                OP.mult)
                    # closs accumulation + coverage update
                    mt = small.tile([128, BL * NCH], F32, tag="mt")
                    nc.vector.tensor_tensor(mt[:], attn_col[:], cov_col[:],
                                            OP.min)
                    nc.vector.tensor_tensor(acc[:], acc[:], mt[:], OP.add)
                    nc.vector.tensor_tensor(cov_col[:], cov_col[:],
                                            attn_col[:], OP.add)
                    nc.vector.tensor_copy(out=covb16[:], in_=cov_col[:])
                    nc.sync.dma_start(out=covT[:], in_=covb16[:],
                                        transpose=True)
                    nc.gpsimd.dma_start(out=d_covrow[:], in_=covT[:])
                    # context vector -> s for next step; outputs
                    nc.vector.tensor_tensor(sT[:], sT_raw[:], recipS_bc[:],
                                            OP.mult)
                    nc.vector.tensor_copy(
                        out=text_acc[:, t * BL:(t + 1) * BL], in_=sT[:])
                    nc.gpsimd.dma_start(out=d_attn[t], in_=attn_col[:])

            nc.sync.dma_start(out=d_acc[:], in_=acc[:])
            nc.sync.dma_start(out=d_text[:], in_=text_acc[:])

    nc.compile()
    return nc


def _prep_inputs(z, h0, c0, W_ih, W_hh, b_ih, b_hh, W_x, W_z, w_c, b_attn, v):
    shared = {
        "wihT": np.ascontiguousarray(np.asarray(W_ih, np.float32).T),
        "whhT": np.ascontiguousarray(np.asarray(W_hh, np.float32).T),
        "wxh": np.ascontiguousarray(np.asarray(W_x, np.float32)[:H]),
        "wxc": np.ascontiguousarray(np.asarray(W_x, np.float32)[H:]),
        "wz": np.ascontiguousarray(np.asarray(W_z, np.float32)),
        "wc": np.asarray(w_c, np.float32).reshape(A, 1),
        "v": np.asarray(v, np.float32).reshape(A, 1),
        "v16": np.asarray(v, np.float16).reshape(A, 1),
        "batt": np.asarray(b_attn, np.float32).reshape(A, 1),
    }
    bgf = (np.asarray(b_ih, np.float32) + np.asarray(b_hh, np.float32))
    bg = np.ascontiguousarray(bgf.reshape(4, H).T)  # [H, 4] cols i,f,g,o
    bg = bg * np.array([0.5, 0.5, 1.0, 0.5], np.float32)[None, :]
    shared["bg"] = np.ascontiguousarray(bg)

    z = np.asarray(z, np.float32)
    h0 = np.asarray(h0, np.float32)
    c0 = np.asarray(c0, np.float32)
    in_maps = []
    for ci in range(NCORES):
        bsl = slice(ci * BL, (ci + 1) * BL)
        zc = z[bsl]                                    # [BL, N, DK]
        zt = np.ascontiguousarray(
            zc.transpose(2, 0, 1).reshape(128, BL * N))
        znb = np.ascontiguousarray(
            zc.reshape(BL, NCH, 128, DK).transpose(2, 0, 1, 3)
              .reshape(128, BL * N)).astype(np.float16)
        m = dict(shared)
        m["zt"] = zt
        m["znb"] = znb
        m["h0t"] = np.ascontiguousarray(h0[bsl].T)
        m["c0t"] = np.ascontiguousarray(c0[bsl].T)
        in_maps.append(m)
    return in_maps


def kernel(z, mask, h0, c0, W_ih, W_hh, b_ih, b_hh, W_x, W_z, w_c, b_attn, v,
           n_node, _trace=False):
    n_steps = int(n_node)
    nc = _build(n_steps)
    in_maps = _prep_inputs(z, h0, c0, W_ih, W_hh, b_ih, b_hh, W_x, W_z,
                           w_c, b_attn, v)
    res = run_bass_kernel_spmd(nc, in_maps, list(range(NCORES)),
                               trace=_trace)
    text = np.empty((B, n_steps, DK), np.float32)
    attns = np.empty((B, n_steps, N), np.float32)
    closs = 0.0
    for ci, r in enumerate(res.results):
        bsl = slice(ci * BL, (ci + 1) * BL)
        to = r["text_out"].reshape(DK, n_steps, BL)        # [d, t, b]
        text[bsl] = to.transpose(2, 1, 0)                  # [BL, T, DK]
        a4 = r["attn_out"].reshape(n_steps, 128, BL, NCH)  # [t, p, b, c]
        attns[bsl] = a4.transpose(2, 0, 3, 1).reshape(BL, n_steps, N)
        closs += r["acc_out"].astype(np.float64).sum()
    closs = np.float32(closs / B)
    kernel._last_result = res
    return text, attns, closs
